# revision 15
# baseline (speedup 1.0000x reference)
"""AttnBlock (GroupNorm + single-head self-attention + residual) on 8 TRN2 cores.

Strategy: data-parallel over batch (16 images -> 2 per core), no collectives.

Key speedups over a pure-fp32r formulation:
  * The q/k projections are folded on the host: s = q.k = h (Wq^T Wk) h^T, so
    the device computes g = M^T h once (M = Wq^T Wk precomputed in f64) and
    scores directly from (g, h). The bq bias becomes a per-channel additive
    term on g (exact); the bk bias shifts every score in a row equally and
    cancels in softmax; bv is folded into the output bias (bp' = bp + Wp bv).
  * The N^2-sized matmuls (scores, context) plus the v/out projections run in
    fp8e4 with DoubleRow perf mode: two 128-deep contraction tiles per pass =
    2x PE throughput. Softmax averaging dilutes the quantization noise; the
    g-projection (whose error would be amplified through exp) stays fp32r.
  * exp is computed as exp(s/sqrt(C) - 4): keeps e under fp8e4's 240 max
    (scores reach ~6.7) and the shift cancels through the softmax denominator,
    which is accumulated from the same shifted fp8 values by tiny ones-matmuls.
  * Softmax normalization is deferred through the linear output projection
    (ctx columns scaled by 1/den at evacuation); the tail is one fused
    (psum + bp') + x op per tile.
  * Phases of the two batches are interleaved (b1's projections fill b0's
    exp/denominator stalls and vice versa), matmul accumulation groups are
    emitted as [128, 2, 512] PSUM pairs so every evacuation is one wide op,
    and batch-0 groupnorm stats are split DVE/ACT (Square+Identity accum_out)
    to shorten the critical startup chain.
"""

import numpy as np

B, C, HW = 16, 512, 1024
H = W = 32
NCORES = 8
BPC = B // NCORES
GROUPS = 32
GSIZE = C // GROUPS  # 16
EPS = 1e-5
SHIFT = 4.0  # exp(s - SHIFT); cancels via the denominator

_CACHE = {}


def _build_nc():
    import concourse.bacc as bacc
    import concourse.tile as tile
    from concourse import mybir

    R = mybir.dt.float32r
    F = mybir.dt.float32
    F8 = mybir.dt.float8e4
    A = mybir.AluOpType
    AF = mybir.ActivationFunctionType
    DR = mybir.MatmulPerfMode.DoubleRow

    nc = bacc.Bacc("TRN2", target_bir_lowering=False, debug=False)

    x = nc.declare_dram_parameter("x", [BPC, C, HW], F, isOutput=False)
    wm = nc.declare_dram_parameter("wm", [C, C], R, isOutput=False)  # M = wq^T wk
    wv8 = nc.declare_dram_parameter("wv8", [C, C], F8, isOutput=False)
    wp8 = nc.declare_dram_parameter("wp8", [C, C], F8, isOutput=False)
    vecs = nc.declare_dram_parameter("vecs", [128, 4, 4], F, isOutput=False)
    gmask = nc.declare_dram_parameter("gmask", [128, 8], F, isOutput=False)
    gmaskT = nc.declare_dram_parameter("gmaskT", [8, 128], F, isOutput=False)
    ones8 = nc.declare_dram_parameter("ones8", [128, 2, 16], F8, isOutput=False)
    ones_row = nc.declare_dram_parameter("ones_row", [1, 128], R, isOutput=False)
    y = nc.declare_dram_parameter("y", [BPC, C, HW], F, isOutput=True)

    with tile.TileContext(nc) as tc:
        import contextlib

        ctx = contextlib.ExitStack()
        with ctx:
            wpool = ctx.enter_context(tc.tile_pool(name="w", bufs=1))
            cpool = ctx.enter_context(tc.tile_pool(name="c", bufs=1))
            xpool = ctx.enter_context(tc.tile_pool(name="x", bufs=2))
            hpool = ctx.enter_context(tc.tile_pool(name="h", bufs=2))
            h8pool = ctx.enter_context(tc.tile_pool(name="h8", bufs=2))
            qpool = ctx.enter_context(tc.tile_pool(name="q", bufs=2))
            vpool = ctx.enter_context(tc.tile_pool(name="v", bufs=2))
            epool = ctx.enter_context(tc.tile_pool(name="e", bufs=2))
            spool = ctx.enter_context(tc.tile_pool(name="s", bufs=2))
            rpool = ctx.enter_context(tc.tile_pool(name="r", bufs=2))
            opool = ctx.enter_context(tc.tile_pool(name="o", bufs=3))
            mpool = ctx.enter_context(tc.tile_pool(name="mp", bufs=3, space="PSUM"))
            gpool = ctx.enter_context(tc.tile_pool(name="gp", bufs=2, space="PSUM"))

            # ---- persistent loads -------------------------------------------
            xts = []
            for b in range(BPC):
                xt_b = xpool.tile([128, 4, HW], F, tag="x", name=f"xt{b}")
                xts.append(xt_b)
            from concourse.tile import add_dep_helper

            xsrc = [x.ap()[b].rearrange("(i p) n -> p i n", p=128) for b in range(BPC)]

            # DMA order = HBM-bandwidth priority order (batch-0 x gates
            # groupnorm stats and the whole pipeline).
            # x0's 8 chunks in parallel queues (aggregate-bandwidth bound);
            # later groups are chained behind it so x0 keeps full bandwidth
            x0_dmas = []
            for i in range(4):
                for s in range(2):
                    d = nc.sync.dma_start(out=xts[0][:, i, s * 512 : (s + 1) * 512],
                                          in_=xsrc[0][:, i, s * 512 : (s + 1) * 512])
                    x0_dmas.append(d)
            gmask_t = cpool.tile([128, 8], F, tag="gmask")
            nc.sync.dma_start(out=gmask_t, in_=gmask.ap())
            gmaskT_t = cpool.tile([8, 128], F, tag="gmaskT")
            nc.sync.dma_start(out=gmaskT_t, in_=gmaskT.ap())
            vecs_t = cpool.tile([128, 4, 4], F, tag="vecs")
            nc.sync.dma_start(out=vecs_t, in_=vecs.ap())
            # pair-dim stride must be a multiple of 16 for DoubleRow ldweights
            ones8_t = cpool.tile([128, 2, 16], F8, tag="ones8")
            nc.sync.dma_start(out=ones8_t, in_=ones8.ap())
            ones_row_t = cpool.tile([1, 128], R, tag="ones_row")
            nc.sync.dma_start(out=ones_row_t, in_=ones_row.ap())
            eps8 = cpool.tile([8, 1], F, tag="eps8")
            nc.vector.memset(eps8, EPS)
            nshift = cpool.tile([128, 1], F, tag="nshift")
            nc.vector.memset(nshift, -SHIFT)
            # dep-free dummy op: pulls the ACT function-table load to t~0
            # instead of serializing it into the groupnorm stats chain
            actwarm = cpool.tile([128, 1], F, tag="actwarm")
            nc.scalar.activation(out=actwarm, in_=nshift, func=AF.Identity)

            # PE warmup: keeps the tensor engine busy (and the clock ramped)
            # while batch-0 x and the groupnorm stats crunch through.
            wrm = cpool.tile([128, 128], F, tag="wrm")
            nc.vector.memset(wrm, 0.0)
            wps = mpool.tile([128, 2, 512], F, tag="mm")

            def warmup(n):
                for j in range(n):
                    nc.tensor.matmul(wps[:, 0, 0:128], wrm, wrm, start=(j == 0),
                                     stop=(j == n - 1))

            warmup(24)

            wm_t = wpool.tile([128, 4, C], R, tag="wm")
            wv8_t = wpool.tile([128, 4, C], F8, tag="wv8")
            wp8_t = wpool.tile([128, 4, C], F8, tag="wp8")
            wmsrc = wm.ap().rearrange("(ct p) o -> p ct o", p=128)
            wv8src = wv8.ap().rearrange("(ct p) o -> p ct o", p=128)
            wp8src = wp8.ap().rearrange("(ct p) o -> p ct o", p=128)
            groups = [
                [(wm_t[:, ct, :], wmsrc[:, ct, :]) for ct in range(4)],
                [(wv8_t[:, 2 * j : 2 * j + 2, :], wv8src[:, 2 * j : 2 * j + 2, :])
                 for j in range(2)],
                [(xts[1][:, i, :], xsrc[1][:, i, :]) for i in range(4)],
                [(wp8_t[:, 2 * j : 2 * j + 2, :], wp8src[:, 2 * j : 2 * j + 2, :])
                 for j in range(2)],
            ]
            prev = x0_dmas[-1]
            for grp in groups:
                ds = []
                for out_ap, in_ap in grp:
                    d = nc.sync.dma_start(out=out_ap, in_=in_ap)
                    add_dep_helper(d.ins, prev.ins, reason="dma bandwidth order")
                    ds.append(d)
                prev = ds[-1]

            # ---- groupnorm for both batches, pipelined per 128-channel tile.
            # Batch 0 splits per-channel stats across ACT (tiles 0-1, via
            # Identity/Square accum_out) and DVE (tiles 2-3, bn_stats) to
            # halve the serial chain that gates the first projection.
            hts = [None, None]
            ht8s = [None, None]

            def groupnorm(b):
                xt = xts[b]
                ht = hpool.tile([128, 4, HW], R, tag="hctx", name=f"ht{b}")
                hts[b] = ht
                ht8 = h8pool.tile([128, 4, HW], F8, tag="h8", name=f"ht8_{b}")
                ht8s[b] = ht8
                varga = spool.tile([8, 4], F, tag="varga")
                sda = spool.tile([8, 4], F, tag="sda")
                ggs = {}

                def finish(i, gg, b=b, xt=xt, ht=ht, ht8=ht8, sda=sda):
                    st2 = spool.tile([8, 2], F, tag=f"st2{i}")
                    with nc.allow_low_precision("groupnorm rstd"):
                        nc.vector.reciprocal(out=st2[:, 0:1], in_=sda[:, i : i + 1])
                    nc.vector.tensor_copy(out=st2[:, 1:2], in_=gg[:, 0:1])
                    bc = gpool.tile([128, 2], F, tag="gn")
                    nc.tensor.matmul(bc, gmaskT_t, st2, start=True, stop=True)
                    scale_c = spool.tile([128, 1], F, tag=f"scale{i}")
                    nc.vector.tensor_mul(out=scale_c, in0=bc[:, 0:1], in1=vecs_t[:, i, 0:1])
                    tmp = spool.tile([128, 1], F, tag=f"tmp{i}")
                    nc.vector.tensor_mul(out=tmp, in0=bc[:, 1:2], in1=scale_c)
                    shift_c = spool.tile([128, 1], F, tag=f"shift{i}")
                    nc.vector.tensor_sub(out=shift_c, in0=vecs_t[:, i, 1:2], in1=tmp)
                    if b == 0 and i < 3:
                        warmup(4)
                    if b == 0:
                        nc.scalar.activation(out=ht[:, i, :], in_=xt[:, i, :],
                                             func=AF.Identity, bias=shift_c,
                                             scale=scale_c)
                    else:
                        # Pool, not DVE: keeps b1's normalize off the DVE
                        # queue, which otherwise head-of-line blocks the
                        # attention evacuations emitted later
                        nc.gpsimd.tensor_scalar(
                            out=ht[:, i, :], in0=xt[:, i, :],
                            scalar1=scale_c, scalar2=shift_c, op0=A.mult, op1=A.add)
                    # fp8 shadow copy for the DoubleRow operands (Pool engine,
                    # SBUF->SBUF; Pool cannot touch PSUM so this is its niche)
                    nc.gpsimd.tensor_scalar(
                        out=ht8[:, i, :], in0=xt[:, i, :],
                        scalar1=scale_c, scalar2=shift_c, op0=A.mult, op1=A.add)

                for i in range(4):
                    stats_i = spool.tile([128, 2], F, tag=f"stats{i}")
                    if b == 0 and i < 2:
                        # ACT path: accumulate sum(x) and sum(x^2) over the
                        # free dim; ht is a scratch destination (rewritten by
                        # the real normalize later on the same engine).
                        sx2 = spool.tile([128, 2], F, tag=f"sx{i}")
                        with nc.allow_low_precision("gn stats accum"):
                            nc.scalar.activation(out=ht[:, i, :], in_=xt[:, i, :],
                                                 func=AF.Identity,
                                                 accum_out=sx2[:, 0:1])
                            nc.scalar.activation(out=ht[:, i, :], in_=xt[:, i, :],
                                                 func=AF.Square,
                                                 accum_out=sx2[:, 1:2])
                        nc.vector.tensor_scalar_mul(out=stats_i, in0=sx2,
                                                    scalar1=1.0 / HW)
                    else:
                        xr = xt[:, i, :].rearrange("p (s d) -> p s d", d=512)
                        st6 = spool.tile([128, 2, 6], F, tag=f"st6{i}")
                        for s in range(2):
                            nc.vector.bn_stats(out=st6[:, s, :], in_=xr[:, s, :])
                        mv = spool.tile([128, 2], F, tag=f"mv{i}")
                        nc.vector.bn_aggr(out=mv, in_=st6)
                        m2c = spool.tile([128, 1], F, tag=f"m2c{i}")
                        nc.vector.tensor_mul(out=m2c, in0=mv[:, 0:1], in1=mv[:, 0:1])
                        nc.vector.tensor_add(out=stats_i[:, 1:2], in0=mv[:, 1:2],
                                             in1=m2c)
                        nc.vector.tensor_copy(out=stats_i[:, 0:1], in_=mv[:, 0:1])
                    gps = gpool.tile([8, 2], F, tag="gn")
                    nc.tensor.matmul(gps, gmask_t, stats_i, start=True, stop=True)
                    gg = spool.tile([8, 2], F, tag=f"gg{i}")
                    ggs[i] = gg
                    nc.vector.tensor_scalar_mul(out=gg, in0=gps, scalar1=1.0 / GSIZE)
                    m2g = spool.tile([8, 1], F, tag=f"m2g{i}")
                    nc.vector.tensor_mul(out=m2g, in0=gg[:, 0:1], in1=gg[:, 0:1])
                    nc.vector.tensor_sub(out=varga[:, i : i + 1], in0=gg[:, 1:2],
                                         in1=m2g)
                    if b == 0:
                        nc.scalar.activation(out=sda[:, i : i + 1],
                                             in_=varga[:, i : i + 1],
                                             func=AF.Sqrt, bias=eps8, scale=1.0)
                        finish(i, gg)
                if b == 1:
                    nc.scalar.activation(out=sda, in_=varga, func=AF.Sqrt,
                                         bias=eps8, scale=1.0)
                    for i in range(4):
                        finish(i, ggs[i])

            # ---- attention phases (emitted interleaved across batches) ------
            gt8s = {}
            vt8s = {}
            et8s = {}
            ct8s = {}
            rbs = {}

            def pair(t):
                return t.rearrange("p (s d) -> p s d", d=512)

            def gproj(b):
                ht = hts[b]
                gt8 = qpool.tile([128, 4, HW], F8, tag="g8", name=f"g8_{b}")
                gt8s[b] = gt8
                for ot in range(4):
                    pp = mpool.tile([128, 2, 512], F, tag="mm", name=f"pj{b}_{ot}")
                    for ct in range(4):
                        for nh in range(2):
                            nc.tensor.matmul(
                                pp[:, nh, :],
                                wm_t[:, ct, ot * 128 : (ot + 1) * 128],
                                ht[:, ct, nh * 512 : (nh + 1) * 512],
                                start=(ct == 0), stop=(ct == 3))
                    # u-bias (Wk^T bq) folded in; fp8 out for DoubleRow
                    nc.scalar.activation(out=pair(gt8[:, ot, :]), in_=pp,
                                         func=AF.Identity,
                                         bias=vecs_t[:, ot, 3:4], scale=1.0)

            def vproj(b):
                ht8 = ht8s[b]
                vt8 = vpool.tile([128, 8, 512], F8, tag="v8", name=f"v8_{b}")
                vt8s[b] = vt8
                for mt in range(8):
                    ps = mpool.tile([128, 2, 512], F, tag="mm")
                    for g in range(2):
                        nc.tensor.matmul(
                            ps[:, 0, :],
                            ht8[:, 2 * g : 2 * g + 2, mt * 128 : (mt + 1) * 128],
                            wv8_t[:, 2 * g : 2 * g + 2, :],
                            start=(g == 0), stop=(g == 1), perf_mode=DR)
                    nc.scalar.copy(out=vt8[:, mt, :], in_=ps[:, 0, :])

            def scores(b):
                ht8 = ht8s[b]
                gt8 = gt8s[b]
                et8 = epool.tile([128, 8, HW], F8, tag="e8", name=f"e8_{b}")
                et8s[b] = et8
                for mt in range(8):
                    pp = mpool.tile([128, 2, 512], F, tag="mm", name=f"sc{b}_{mt}")
                    for g in range(2):
                        for nh in range(2):
                            nc.tensor.matmul(
                                pp[:, nh, :],
                                ht8[:, 2 * g : 2 * g + 2, mt * 128 : (mt + 1) * 128],
                                gt8[:, 2 * g : 2 * g + 2, nh * 512 : (nh + 1) * 512],
                                start=(g == 0), stop=(g == 1), perf_mode=DR)
                    nc.scalar.activation(out=pair(et8[:, mt, :]), in_=pp,
                                         func=AF.Exp, scale=float(C ** -0.5),
                                         bias=nshift)

            def denctx(b):
                vt8 = vt8s[b]
                et8 = et8s[b]
                rc = rpool.tile([1, HW], R, tag="recip", name=f"rc{b}")
                rb_sb = rpool.tile([128, 2, 512], F, tag="rb", name=f"rb{b}")
                rbs[b] = rb_sb
                ct8 = h8pool.tile([128, 4, HW], F8, tag="h8", name=f"ct8_{b}")
                ct8s[b] = ct8
                psd = [gpool.tile([1, 512], F, tag="gn", name=f"psd{b}_{nh}")
                       for nh in range(2)]
                # softmax denominator from the same shifted fp8 e tiles
                for nh in range(2):
                    for g in range(4):
                        nc.tensor.matmul(
                            psd[nh], ones8_t[:, :, 0:1],
                            et8[:, 2 * g : 2 * g + 2, nh * 512 : (nh + 1) * 512],
                            start=(g == 0), stop=(g == 3), perf_mode=DR)
                for nh in range(2):
                    nc.scalar.copy(out=rc[:, nh * 512 : (nh + 1) * 512],
                                   in_=psd[nh])
                    prb = gpool.tile([128, 512], F, tag="gn")
                    nc.tensor.matmul(prb, ones_row_t,
                                     rc[0:1, nh * 512 : (nh + 1) * 512],
                                     start=True, stop=True)
                    nc.vector.reciprocal_approx_fast(out=rb_sb[:, nh, :], in_=prb)
                for c2 in range(4):
                    pp = mpool.tile([128, 2, 512], F, tag="mm", name=f"cx{b}_{c2}")
                    for g in range(4):
                        for nh in range(2):
                            nc.tensor.matmul(
                                pp[:, nh, :],
                                vt8[:, 2 * g : 2 * g + 2, c2 * 128 : (c2 + 1) * 128],
                                et8[:, 2 * g : 2 * g + 2, nh * 512 : (nh + 1) * 512],
                                start=(g == 0), stop=(g == 3), perf_mode=DR)
                    # deferred softmax normalization folded into the evac
                    nc.vector.tensor_mul(out=pair(ct8[:, c2, :]), in0=pp,
                                         in1=rb_sb)

            def outproj(b):
                xt = xts[b]
                ct8 = ct8s[b]
                for pt in range(4):
                    pp = mpool.tile([128, 2, 512], F, tag="mm", name=f"yp{b}_{pt}")
                    for g in range(2):
                        for nh in range(2):
                            nc.tensor.matmul(
                                pp[:, nh, :],
                                wp8_t[:, 2 * g : 2 * g + 2, pt * 128 : (pt + 1) * 128],
                                ct8[:, 2 * g : 2 * g + 2, nh * 512 : (nh + 1) * 512],
                                start=(g == 0), stop=(g == 1), perf_mode=DR)
                    o_t = opool.tile([128, HW], F, tag="o1")
                    for nh in range(2):
                        nc.vector.scalar_tensor_tensor(
                            out=o_t[:, nh * 512 : (nh + 1) * 512], in0=pp[:, nh, :],
                            scalar=vecs_t[:, pt, 2:3],
                            in1=xt[:, pt, nh * 512 : (nh + 1) * 512],
                            op0=A.add, op1=A.add)
                        nc.sync.dma_start(
                            out=y.ap()[b][pt * 128 : (pt + 1) * 128,
                                          nh * 512 : (nh + 1) * 512],
                            in_=o_t[:, nh * 512 : (nh + 1) * 512])

            # b1 phases slotted into b0's exp/denominator shadows (the PE
            # executes in emission order, so these fill what would otherwise
            # be head-of-line stalls on the ACT exp stream). b1's groupnorm is
            # emitted after b0's projections: its stats wait on the x1 DMA,
            # and emitting them earlier head-of-line blocks the DVE queue.
            groupnorm(0)
            gproj(0)
            vproj(0)
            groupnorm(1)
            scores(0)
            gproj(1)
            denctx(0)
            vproj(1)
            scores(1)
            outproj(0)
            denctx(1)
            outproj(1)

    nc.finalize()
    return nc


def _get_nc():
    if "nc" not in _CACHE:
        _CACHE["nc"] = _build_nc()
    return _CACHE["nc"]


def make_in_maps(inputs):
    import ml_dtypes

    E4 = ml_dtypes.float8_e4m3
    x = np.asarray(inputs["x"], np.float32).reshape(B, C, HW)
    f32 = lambda a: np.ascontiguousarray(np.asarray(a, np.float32))
    f64 = lambda a: np.asarray(a, np.float64)
    # M = wq^T wk so that s_nm = h_n^T M h_m  (folds the q/k projections)
    M = (f64(inputs["wq"]).T @ f64(inputs["wk"])).astype(np.float32)
    # u = wk^T bq: the only bq term that survives softmax; added onto g
    u = (f64(inputs["wk"]).T @ f64(inputs["bq"])).astype(np.float32)
    # bv folded into the output bias: o = Wp(ctx) + (bp + Wp bv)
    bpp = (f64(inputs["bp"]) + f64(inputs["wp"]) @ f64(inputs["bv"])).astype(np.float32)
    wvT8 = f32(inputs["wv"]).T.copy().astype(E4)
    wpT8 = f32(inputs["wp"]).T.copy().astype(E4)
    vstack = np.stack([f32(inputs["gn_w"]), f32(inputs["gn_b"]), bpp, u])  # [4, C]
    vecs = np.ascontiguousarray(vstack.reshape(4, 4, 128).transpose(2, 1, 0))
    gmask = np.zeros((128, 8), np.float32)
    for p in range(128):
        gmask[p, p // GSIZE] = 1.0
    gmaskT = gmask.T.copy()
    ones8 = np.ones((128, 2, 16), np.float32).astype(E4)
    ones_row = np.ones((1, 128), np.float32)

    shared = {"wm": M, "wv8": wvT8, "wp8": wpT8, "vecs": vecs,
              "gmask": gmask, "gmaskT": gmaskT, "ones8": ones8,
              "ones_row": ones_row}
    return [dict(shared, x=np.ascontiguousarray(x[i * BPC : (i + 1) * BPC]))
            for i in range(NCORES)]


def kernel(**inputs) -> np.ndarray:
    from concourse.bass_utils import run_bass_kernel_spmd

    core_ids = list(range(NCORES))
    in_maps = make_in_maps(inputs)
    nc = _get_nc()
    res = run_bass_kernel_spmd(nc, in_maps, core_ids)
    out = np.concatenate([res.results[i]["y"] for i in core_ids], axis=0)
    return out.reshape(B, C, H, W)
